# revision 1
# baseline (speedup 1.0000x reference)
"""Trainium2 Bass kernel for Llama-style GQA causal self-attention.

Problem (hardcoded): T=4096, HID=2048, D=128, NQ=16, NKV=4, rotate-half RoPE,
causal softmax, o_proj.  8 NeuronCores, tensor-parallel over heads:

  core c: Q heads {2c, 2c+1}, KV head c//2.
  phase 1: QKV projections from pre-transposed bf16 hidden states
           (qT/kT layout [d, t]; v layout [t, d]), RoPE on DVE.
  phase 2: causal attention in S^T orientation:
           S^T[k,q] = kT.T @ qT chunks, exp on ScalarE (no max subtraction --
           logits are O(3) by construction), causal zero-fill via GpSimd
           affine_select, PV as outT[d,q] += v_chunk.T @ P^T, softmax sums
           via ones-matmul, normalize via broadcasted reciprocal.
  phase 3: per-head AllToAll heads->sequence reshard (1MB each), overlapped:
           head-0 A2A runs during head-1 attention.
  phase 4: o_proj in two half-contractions (head-0 chunks, then head-1
           chunks DMA-accumulated into the output).

Host-side prep is layout only: transpose + bf16-cast of weights/activations,
RoPE cos/sin tables from position_ids, sharding, final concat.
"""

import numpy as np
import ml_dtypes

import concourse.bass as bass
import concourse.mybir as mybir
import concourse.tile as tile
from concourse import bacc
from concourse.bass_utils import run_bass_kernel_spmd
from concourse.masks import make_identity

T, HID, D = 4096, 2048, 128
NQ, NKV = 16, 4
THETA = 10000.0
NCORES = 8
HPC = NQ // NCORES        # q heads per core = 2
TB = 512                  # t block
NT = T // TB              # 8
KC = HID // 128           # 16 contraction chunks
HALF = D // 2             # 64
SCALE = 1.0 / float(np.sqrt(D))
FP32 = mybir.dt.float32
BF16 = mybir.dt.bfloat16
NPBF16 = ml_dtypes.bfloat16


def _build_nc():
    nc = bacc.Bacc("TRN2", num_devices=NCORES)

    hsT = nc.declare_dram_parameter("hsT", [HID, T], BF16, isOutput=False)
    wqT = nc.declare_dram_parameter("wqT", [HID, HPC * D], BF16, isOutput=False)
    wkT = nc.declare_dram_parameter("wkT", [HID, D], BF16, isOutput=False)
    wvT = nc.declare_dram_parameter("wvT", [HID, D], BF16, isOutput=False)
    woT = nc.declare_dram_parameter("woT", [HID, HID], BF16, isOutput=False)
    cosT = nc.declare_dram_parameter("cosT", [D, T], BF16, isOutput=False)
    sinT = nc.declare_dram_parameter("sinT", [D, T], BF16, isOutput=False)
    outp = nc.declare_dram_parameter("out", [TB, HID], FP32, isOutput=True)

    # per-head collective bounce buffers (internal DRAM)
    a2a_in = [nc.dram_tensor(f"a2a_in{h}", [NCORES, D, TB], BF16) for h in range(HPC)]
    a2a_out = [nc.dram_tensor(f"a2a_out{h}", [NCORES, D, TB], BF16) for h in range(HPC)]

    hsT_r = hsT.rearrange("(c p) t -> p c t", p=128)
    wqT_r = wqT.rearrange("(c p) m -> p c m", p=128)
    wkT_r = wkT.rearrange("(c p) m -> p c m", p=128)
    wvT_r = wvT.rearrange("(c p) m -> p c m", p=128)
    woT_r = woT.rearrange("(c p) m -> p c m", p=128)

    with tile.TileContext(nc) as tc:
        with (
            tc.tile_pool(name="const", bufs=1) as cpool,
            tc.tile_pool(name="hsx", bufs=2) as hpool,
            tc.tile_pool(name="qkv", bufs=1) as qpool,
            tc.tile_pool(name="pt", bufs=6) as ptpool,
            tc.tile_pool(name="tmp", bufs=4) as tpool,
            tc.tile_pool(name="rec", bufs=1) as rpool,
            tc.tile_pool(name="ps", bufs=4, space="PSUM") as ps,
            tc.tile_pool(name="acc", bufs=2, space="PSUM") as psacc,
            tc.tile_pool(name="lsum", bufs=2, space="PSUM") as pslsum,
        ):
            # ---- constants / weights (wo loaded later, it is only needed in
            # phase 4 and its 8.4MB would delay the first hsT block) ----
            wq_sb = cpool.tile([128, KC, HPC * D], BF16, tag="wq")
            wk_sb = cpool.tile([128, KC, D], BF16, tag="wk")
            wv_sb = cpool.tile([128, KC, D], BF16, tag="wv")
            cos_sb = cpool.tile([D, T], BF16, tag="cos")
            sin_sb = cpool.tile([D, T], BF16, tag="sin")
            ones_sb = cpool.tile([128, 1], BF16, tag="ones")
            for c4 in range(4):  # split across DMA queues
                cs = slice(c4 * 4, (c4 + 1) * 4)
                nc.sync.dma_start(out=wq_sb[:, cs, :], in_=wqT_r[:, cs, :])
            nc.gpsimd.memset(ones_sb[:, :], 1.0)
            ident = cpool.tile([128, 128], BF16, tag="ident")
            make_identity(nc, ident[:, :])

            qT = qpool.tile([128, HPC, T], BF16, tag="qT")
            kT = qpool.tile([128, T], BF16, tag="kT")
            vv = qpool.tile([128, T // 128, D], BF16, tag="vv")
            attnT = qpool.tile([128, HPC, T], BF16, tag="attnT")

            def rope(dst, src_psum, cos_b, sin_b):
                # dst = src*cos + rotate_half(src)*sin, rotate_half = [-x2; x1]
                qw = tpool.tile([128, TB], BF16, tag="projw")
                rot = tpool.tile([128, TB], BF16, tag="projw")
                nc.scalar.copy(qw[:, :], src_psum[:, :])
                nc.scalar.activation(
                    rot[0:HALF, :], src_psum[HALF:128, :],
                    mybir.ActivationFunctionType.Copy, scale=-1.0,
                )
                nc.scalar.copy(rot[HALF:128, :], src_psum[0:HALF, :])
                ta = tpool.tile([128, TB], BF16, tag="ropetmp")
                tb_ = tpool.tile([128, TB], BF16, tag="ropetmp")
                nc.vector.tensor_tensor(ta, qw, cos_b, mybir.AluOpType.mult)
                nc.vector.tensor_tensor(tb_, rot, sin_b, mybir.AluOpType.mult)
                nc.vector.tensor_tensor(dst, ta, tb_, mybir.AluOpType.add)

            # ---- phase 1: projections + RoPE ----
            for tb in range(NT):
                ts = slice(tb * TB, (tb + 1) * TB)
                hsx = hpool.tile([128, KC, TB], BF16, tag="hsx")
                for c4 in range(4):  # split across DMA queues
                    cs = slice(c4 * 4, (c4 + 1) * 4)
                    nc.sync.dma_start(out=hsx[:, cs, :], in_=hsT_r[:, cs, ts])
                if tb == 0:
                    # secondary weights, after the critical wq+hsx0 loads
                    nc.sync.dma_start(out=wk_sb[:, :, :], in_=wkT_r)
                    nc.sync.dma_start(out=wv_sb[:, :, :], in_=wvT_r)
                    nc.sync.dma_start(out=cos_sb[:, :], in_=cosT[:, :])
                    nc.sync.dma_start(out=sin_sb[:, :], in_=sinT[:, :])
                cos_b = cos_sb[:, ts]
                sin_b = sin_sb[:, ts]

                for h in range(HPC):
                    qps = ps.tile([128, TB], FP32, tag="mm512")
                    for c in range(KC):
                        nc.tensor.matmul(
                            qps[:, :],
                            lhsT=wq_sb[:, c, h * D:(h + 1) * D],
                            rhs=hsx[:, c, :],
                            start=(c == 0), stop=(c == KC - 1),
                        )
                    rope(qT[:, h, ts], qps, cos_b, sin_b)

                kps = ps.tile([128, TB], FP32, tag="mm512")
                for c in range(KC):
                    nc.tensor.matmul(
                        kps[:, :], lhsT=wk_sb[:, c, :], rhs=hsx[:, c, :],
                        start=(c == 0), stop=(c == KC - 1),
                    )
                rope(kT[:, ts], kps, cos_b, sin_b)

                # v computed transposed ([d, t], like kT: big moving dim),
                # then flipped to natural [t, d] on the PE
                vps = ps.tile([128, TB], FP32, tag="mm512")
                for c in range(KC):
                    nc.tensor.matmul(
                        vps[:, :], lhsT=wv_sb[:, c, :], rhs=hsx[:, c, :],
                        start=(c == 0), stop=(c == KC - 1),
                    )
                vtw = tpool.tile([128, TB], BF16, tag="projw")
                nc.vector.tensor_copy(vtw[:, :], vps[:, :])
                vtp = pslsum.tile([128, TB], BF16, tag="lsum")
                for tt in range(TB // 128):
                    nc.tensor.transpose(
                        vtp[:, tt * 128:(tt + 1) * 128],
                        vtw[:, tt * 128:(tt + 1) * 128],
                        ident[:, :],
                    )
                nc.vector.tensor_copy(vv[:, tb * 4:(tb + 1) * 4, :], vtp[:, :])

            # ---- phase 2+3: attention per head, A2A per head ----
            wo_sb = cpool.tile([128, KC, HID], BF16, tag="wo")
            for h in range(HPC):
                for i4 in range(NT):
                    qs_full = slice(i4 * TB, (i4 + 1) * TB)
                    nj = 4 * i4 + 4
                    po = psacc.tile([128, TB], FP32, tag="acc")
                    pl = pslsum.tile([1, TB], FP32, tag="lsum")
                    # diagonal (masked) chunks first: their affine_select
                    # latency hides in pipeline fill instead of block tail
                    jorder = list(range(4 * i4, nj)) + list(range(4 * i4))
                    for jpos, j in enumerate(jorder):
                        m = j - 4 * i4  # >=0 on diagonal 512-block
                        off = 128 * m if m > 0 else 0
                        w = TB - off
                        qs = slice(i4 * TB + off, (i4 + 1) * TB)
                        sps = ps.tile([128, TB], FP32, tag="mm512")
                        nc.tensor.matmul(
                            sps[:, 0:w],
                            lhsT=kT[:, j * 128:(j + 1) * 128],
                            rhs=qT[:, h, qs],
                            start=True, stop=True,
                        )
                        pt = ptpool.tile([128, TB], BF16, tag="pt")
                        nc.scalar.activation(
                            pt[:, 0:w], sps[:, 0:w],
                            mybir.ActivationFunctionType.Exp, scale=SCALE,
                        )
                        if m >= 0:
                            # zero entries where q < k (within-block causality)
                            nc.gpsimd.affine_select(
                                out=pt[:, 0:w], in_=pt[:, 0:w],
                                compare_op=mybir.AluOpType.is_ge,
                                fill=0.0, base=0,
                                pattern=[[1, w]], channel_multiplier=-1,
                            )
                        nc.tensor.matmul(
                            po[:, off:TB], lhsT=vv[:, j, :], rhs=pt[:, 0:w],
                            start=(jpos == 0), stop=(jpos == nj - 1),
                        )
                        nc.tensor.matmul(
                            pl[:, off:TB], lhsT=ones_sb[:, :], rhs=pt[:, 0:w],
                            start=(jpos == 0), stop=(jpos == nj - 1),
                        )
                    # normalize: l -> SBUF, broadcast, wide reciprocal, mult
                    ls = rpool.tile([1, TB], FP32, tag="recl")
                    lb = rpool.tile([128, TB], FP32, tag="recb")
                    rb = rpool.tile([128, TB], FP32, tag="recr")
                    scr = rpool.tile([128, TB], FP32, tag="recs")
                    nc.scalar.copy(ls[:, :], pl[:, :])
                    nc.gpsimd.partition_broadcast(lb[:, :], ls[0:1, :])
                    nc.vector.reciprocal_approx_accurate(
                        out=rb[:, :], in_=lb[:, :], scratch=scr[:, :]
                    )
                    nc.vector.tensor_tensor(
                        attnT[:, h, qs_full], po[:, :], rb[:, :],
                        mybir.AluOpType.mult,
                    )
                    nc.sync.dma_start(
                        out=a2a_in[h][i4, :, :],
                        in_=attnT[:, h, qs_full],
                    )
                nc.gpsimd.collective_compute(
                    "AllToAll",
                    mybir.AluOpType.bypass,
                    replica_groups=[list(range(NCORES))],
                    ins=[a2a_in[h][:, :, :]],
                    outs=[a2a_out[h][:, :, :]],
                )
                if h == 0:
                    # load wo during head-1 attention
                    nc.sync.dma_start(out=wo_sb[:, :, :], in_=woT_r)

            # ---- phase 4: o_proj rows, one half-contraction per head ----
            for h in range(HPC):
                af = hpool.tile([128, NCORES, TB], BF16, tag="hsx")
                nc.sync.dma_start(
                    out=af[:, :, :],
                    in_=a2a_out[h].rearrange("r p t -> p r t"),
                )
                for tt in range(TB // 128):
                    for oo in range(HID // TB):
                        ops_ = ps.tile([128, TB], FP32, tag="mm512")
                        for r in range(NCORES):
                            nc.tensor.matmul(
                                ops_[:, :],
                                lhsT=af[:, r, tt * 128:(tt + 1) * 128],
                                rhs=wo_sb[:, 2 * r + h, oo * TB:(oo + 1) * TB],
                                start=(r == 0), stop=(r == NCORES - 1),
                            )
                        osb = tpool.tile([128, TB], FP32, tag="osb")
                        nc.vector.tensor_copy(osb[:, :], ops_[:, :])
                        dst = outp[tt * 128:(tt + 1) * 128, oo * TB:(oo + 1) * TB]
                        if h == 0:
                            nc.sync.dma_start(out=dst, in_=osb[:, :])
                        else:
                            nc.gpsimd.dma_start(
                                out=dst, in_=osb[:, :],
                                accum_op=mybir.AluOpType.add,
                            )

    nc.finalize()
    return nc


_NC_CACHE = {}


def _get_nc():
    if "nc" not in _NC_CACHE:
        _NC_CACHE["nc"] = _build_nc()
    return _NC_CACHE["nc"]


def _prep_inputs(hidden_states, wq, wk, wv, wo, position_ids):
    hs = np.asarray(hidden_states, dtype=np.float32)
    hsT = np.ascontiguousarray(hs.T).astype(NPBF16)

    inv_freq = 1.0 / (THETA ** (np.arange(0, HALF, dtype=np.float32) / HALF))
    freqs = np.asarray(position_ids).astype(np.float32)[:, None] * inv_freq[None, :]
    cos1 = np.cos(freqs).T  # [64, T]
    sin1 = np.sin(freqs).T
    cosT = np.ascontiguousarray(np.concatenate([cos1, cos1], axis=0)).astype(NPBF16)
    sinT = np.ascontiguousarray(np.concatenate([sin1, sin1], axis=0)).astype(NPBF16)

    woT = np.ascontiguousarray(np.asarray(wo, dtype=np.float32).T).astype(NPBF16)

    in_maps = []
    for c in range(NCORES):
        kv = c // 2
        wq_c = np.asarray(wq, dtype=np.float32)[2 * c * D:(2 * c + HPC) * D, :]
        in_maps.append({
            "hsT": hsT,
            "wqT": np.ascontiguousarray(wq_c.T).astype(NPBF16),
            "wkT": np.ascontiguousarray(
                np.asarray(wk, dtype=np.float32)[kv * D:(kv + 1) * D, :].T
            ).astype(NPBF16),
            "wvT": np.ascontiguousarray(
                np.asarray(wv, dtype=np.float32)[kv * D:(kv + 1) * D, :].T
            ).astype(NPBF16),
            "woT": woT,
            "cosT": cosT,
            "sinT": sinT,
        })
    return in_maps


def run(inputs, trace=False, tmpdir=None):
    """Run on HW; returns (output, BassKernelResults)."""
    nc = _get_nc()
    in_maps = _prep_inputs(**inputs)
    res = run_bass_kernel_spmd(
        nc, in_maps, core_ids=list(range(NCORES)), trace=trace, tmpdir=tmpdir
    )
    out = np.concatenate(
        [np.asarray(res.results[c]["out"], dtype=np.float32) for c in range(NCORES)],
        axis=0,
    )
    return out, res


def kernel(hidden_states, wq, wk, wv, wo, position_ids):
    out, _ = run(dict(
        hidden_states=hidden_states, wq=wq, wk=wk, wv=wv, wo=wo,
        position_ids=position_ids,
    ))
    return out

